# revision 43
# baseline (speedup 1.0000x reference)
"""Trainium2 Bass kernel for AlignOnlySubLayer.

Per batch b:
    W[c,m]   = sum_d context[b,c,d] * main[b,m,d]
    A        = softmax(W, axis=m)
    out[m,d] = main[b,m,d] - sum_c A[c,m] * context[b,c,d]

Sharding: data-parallel over batch B=8 across the 8 NeuronCores (one batch
per core, no cross-core communication).

Kernel design (per core):
  - ACT is the algorithmic wall: 4M exps at 1 elem/lane/cycle @1.2GHz ~= 2
    x 1.1us half-row ACTIVATEs per c-tile.  Everything else is scheduled
    to keep that chain dense from first exp (~15us) to the end.
  - fp16 inputs, cast on the HOST: the kernel's first on-device step was
    an f32->fp16 cast anyway, so this is numerically identical while
    halving the HBM loads (2MB -> 1MB) and deleting the cast stage.
  - Linear DMA tiling: c/m tile j = rows {16p + j} (partition p = row//16)
    instead of row%128, so per-partition DMA runs are contiguous.
  - Loads: everything exp(0)-critical rides SWDGE in deadline order (ctx
    tiles 0-1, main q0..q3, ctx q1-3) -- across runs SWDGE lands its
    first transfer with low variance while either HW ring can crawl.
    ACT's queue carries no DMA issues before the exp chain.
  - PE pre-warm: dummy matmuls while loads are in flight flip the HAM
    clock gate to 2.4GHz before the real transposes + mm1(0); small
    bursts between batches keep the MID window from re-throttling.
  - Both matmuls run fp16/bf16 (PE 1 col/cycle) with f32 PSUM.  mm1 fp16
    (mantissa), mm2 bf16 (E reaches ~e^70, needs f32 exponent range).
  - d-major operands: main q0-3 + ctx tiles 0-3 transposed on PE (staged
    through the acc PSUM tiles pre-init, ACT/DVE split evacuation); ctx
    q1-3 ride the sync DMA xbar in-loop.
  - acc is FOUR per-m-quarter PSUM tiles: the Tile scheduler serializes
    cross-engine readers of one PSUM tile, so a single acc tile would
    serialize the prologue staging evacs and the tail quarter evacs.
    Separate tiles restore ACT||DVE parallelism, and each tail evac gates
    on its own quarter's final mm2 blocks only.
  - acc is initialized with +main via a PE identity-matmul pass (start=
    True), and the softmax normalization is folded into ctx with a
    NEGATED scale (ctx_s = -context/S), so acc accumulates main-weighted
    directly: the tail needs only a PSUM->SBUF copy, no subtract pass.
  - Software pipeline, one c-tile lookahead: iteration ct emits
    exp(ct)h0, mm2(ct-1)[0..7], mm1(ct+1)h0, exp(ct)h1+accum,
    mm2(ct-1)[8..15], mm1(ct+1)h1, stats(ct).  The two psum_w slots
    alternate h0/h1 so mm1(ct+1)h0 only waits on exp(ct)h0.
  - Row sums: h0 reduced on DVE (off ACT's critical path), h1 via ACT's
    fused accum_out; s = -(h0+h1) and reciprocal give the negated scale.
  - Tail: store rings pre-warmed with tiny WAW-ordered writes two periods
    early; quarter evacs run as ACT||DVE pairs; stores on sync + gpsimd.
"""

import numpy as np

import concourse.bass as bass
import concourse.mybir as mybir
from concourse import bacc
from concourse.masks import make_identity
from concourse.tile import TileContext

P = 128
F32 = mybir.dt.float32
F16 = mybir.dt.float16
BF16 = mybir.dt.bfloat16
EXP = mybir.ActivationFunctionType.Exp
AX = mybir.AxisListType.X
ADD = mybir.AluOpType.add
MULT = mybir.AluOpType.mult
N_CORES = 8


def build_nc(S=2048, D=128, num_devices=N_CORES, repeats=1, precise=False):
    """Build the single-core Bass program (SPMD across cores)."""
    assert D == P and S % P == 0
    T = S // P            # tiles along c (and m); tile j = rows {T*p + j}
    NQ = 4                # quarters (load/store/transpose granularity)
    QT = T // NQ          # tiles per quarter
    QW = QT * P           # f32 elements per partition per quarter
    HALF = S // 2         # columns per mm1 psum half
    MMN = 512             # mm1 moving-operand chunk

    nc = bacc.Bacc(
        "TRN2",
        target_bir_lowering=False,
        debug=False,
        enable_asserts=False,
        num_devices=num_devices,
    )
    # fp16 inputs (cast from f32 on the HOST): the kernel's first on-device
    # step was an f32->fp16 cast anyway, so feeding fp16 is numerically
    # identical while halving the HBM load bytes (2MB -> 1MB) and deleting
    # the whole DVE cast stage from the prologue critical path.
    ctx_d = nc.dram_tensor("context", [S, D], F16, kind="ExternalInput").ap()
    main_d = nc.dram_tensor("main", [S, D], F16, kind="ExternalInput").ap()
    # bf16 output (cast to f32 on host): halves store bytes+descriptors,
    # costs ~2e-3 relative rounding on top of the ~1.8e-3 fp16-pipeline
    # error -- far inside the 2e-2 gate.
    out_d = nc.dram_tensor("out", [S, D], BF16, kind="ExternalOutput").ap()

    # Linear views: partition p <-> rows [T*p, T*p+T), 4KB contiguous each.
    ctx_lin = ctx_d.rearrange("(p r) d -> p (r d)", p=P)
    main_lin = main_d.rearrange("(p r) d -> p (r d)", p=P)
    out_lin = out_d.rearrange("(p r) d -> p (r d)", p=P)

    with TileContext(nc) as tc:
      for _rep in range(repeats):
        with (
            tc.tile_pool(name="persist", bufs=1) as persist,
            tc.tile_pool(name="etile", bufs=4) as etile_pool,
            tc.tile_pool(name="small", bufs=4) as small,
            tc.tile_pool(name="tailp", bufs=4) as tailp,
            tc.tile_pool(name="psum_w", bufs=2, space="PSUM") as psum_w,
            tc.tile_pool(name="psum_acc", bufs=1, space="PSUM") as psum_acc,
        ):
            # ---- persistent SBUF tensors ----
            ctx_h = persist.tile([P, T, P], F16)     # [c_in, j, d]
            main_h = persist.tile([P, T, P], F16)    # [m_in, j, d]
            ctxT = persist.tile([P, T, P], F16)      # [d, j, c_in]
            mainT = persist.tile([P, T, P], F16)     # [d, j, m_in]
            mainT2 = mainT.rearrange("p a b -> p (a b)")
            ident = persist.tile([P, P], F16)

            # Warm the ACT exp table so the ~2.7us load overlaps the DMAs.
            warm = small.tile([P, 1], F32, tag="warm")
            nc.vector.memset(warm[:], 0.0)
            nc.scalar.activation(warm[:], warm[:], EXP)

            def q2(ap3, q):
                return ap3[:, q * QT:(q + 1) * QT].rearrange("p a b -> p (a b)")

            # ---- prologue loads (3 DMA queues: sync/scalar HWDGE rings
            # + gpsimd SWDGE; issue-to-first-packet is ~1.5-2.7us).
            def ldq(eng, raw, lin, q):
                eng.dma_start(q2(raw, q), lin[:, q * QW:(q + 1) * QW],
                              single_packet=True)

            def ldh(eng, raw, lin, t0_, nt):
                eng.dma_start(
                    raw[:, t0_:t0_ + nt].rearrange("p a b -> p (a b)"),
                    lin[:, t0_ * P:(t0_ + nt) * P],
                    single_packet=True,
                )

            # EVERYTHING exp(0)-critical rides SWDGE in deadline order
            # (ctx tiles 0-1, then main q0..q3, then ctx q1): across runs
            # SWDGE lands its first transfer at ~10.5-12us with low
            # variance, while either HW ring can crawl on a given run.
            # In fp16 that whole stream is only 768KB, landing by ~13us.
            # Only slack-rich ctx quarters ride the lottery-prone HW
            # rings (ctx23/ctxq2 feed mm1(2)/mm1(8); ctxq3 feeds mm1(12)).
            # The in-loop ctxT DMA-transposes have opaque write patterns,
            # so every later mm1 conservatively waits on ALL prior dmaTs;
            # a late ctx-quarter load therefore stalls the exp chain from
            # tile 5 on.  Keeping ctx q1-3 on SWDGE bounds every dmaT
            # gate at ~14.5us even on a slow draw.
            ldh(nc.gpsimd, ctx_h, ctx_lin, 0, 2)      # ctx tiles 0-1
            ldq(nc.gpsimd, main_h, main_lin, 0)
            # identity built between the first SWDGE issues: gpsimd is
            # idle there, and the first PE transpose needs ident by ~11us.
            make_identity(nc, ident[:])
            ldq(nc.gpsimd, main_h, main_lin, 1)
            ldq(nc.gpsimd, main_h, main_lin, 2)
            ldq(nc.gpsimd, main_h, main_lin, 3)
            ldq(nc.gpsimd, ctx_h, ctx_lin, 1)
            ldh(nc.sync, ctx_h, ctx_lin, 2, 2)        # ctx tiles 2-3
            ldq(nc.gpsimd, ctx_h, ctx_lin, 2)
            ldq(nc.gpsimd, ctx_h, ctx_lin, 3)

            # ---- PE pre-warm: the HAM clock gate holds the PE at 1.2GHz
            # until it has seen ~3.4us of sustained matmul activity.  The
            # PE is idle while the loads are in flight, so a burst of
            # dummy N=128 matmuls (into PSUM scratch that the acc-init
            # pass later overwrites with start=True) flips it to 2.4GHz
            # before the real transposes + mm1(0) -- halving the cold
            # prologue PE chain.  Garbage values are fine: the dst region
            # is start=True-overwritten before any real read.
            dummy = small.tile([P, 2 * P], F16, tag="dummy")
            nc.vector.memset(dummy[:], 0.0)
            # store-ring warmers: tiny bf16 scratch written into regions
            # the real stores fully overwrite (WAW on the same ring FIFO
            # keeps ordering), so the tail stores skip the ~2.5us DMA
            # cold-start.
            dwarm = small.tile([P, 16], BF16, tag="dwarm")
            nc.vector.memset(dwarm[:], 0.0)

            # ---- casts + d-major transposes ----
            # The out accumulator is FOUR per-m-quarter PSUM tiles (1 bank
            # each) instead of one [P, T, P] tile: the Tile scheduler
            # serializes cross-engine readers of a single PSUM tile, so
            # with one acc tile the prologue staging evacs and the tail
            # quarter evacs all run strictly serially across ACT+DVE.
            # Separate tiles restore ACT||DVE parallelism.
            accq_t = [psum_acc.tile([P, QT, P], F32, tag=f"acc{q}",
                                    name=f"accq{q}")
                      for q in range(NQ)]
            # f16 staging views (pre-acc-init scratch; region q0..q3 hold
            # the transpose staging for main quarter q, the ctx pairs ride
            # the upper halves of tiles 0/1)
            acc16q = [t.rearrange("p a b -> p (a b)").bitcast(F16)
                      for t in accq_t]

            def pe_transpose_batch(nat, dstT, ts, stage, soff, evac=None):
                nt = ts.stop - ts.start
                st = acc16q[stage]
                for i in range(nt):
                    nc.tensor.transpose(
                        st[:, soff + i * P:soff + (i + 1) * P],
                        nat[:, ts.start + i], ident[:],
                    )
                dst = dstT[:, ts].rearrange("p a b -> p (a b)")
                src = st[:, soff:soff + nt * P]
                if evac == "scalar":
                    # ACT is idle before the exp chain and executes in
                    # emission order, immune to the DVE scheduler
                    # reordering evacs behind casts
                    nc.scalar.copy(dst, src)
                else:
                    nc.vector.tensor_copy(dst, src)

            # ---- matmul helpers ----
            w_tiles = {}

            def emit_mm1(ct, h):
                w = psum_w.tile([P, HALF], F32, tag="w")
                w_tiles[(ct, h)] = w
                for j in range(0, HALF, MMN):
                    nc.tensor.matmul(
                        w[:, j:j + MMN],
                        ctxT[:, ct],
                        mainT2[:, h * HALF + j: h * HALF + j + MMN],
                        start=True,
                        stop=True,
                    )

            def emit_ident(qs):
                # acc := +main (exact fp16 copy through the PE so PSUM
                # has_written bits are set for the accumulation group).
                # One N=512 matmul per 2KB PSUM bank, start=True zeroing it.
                for q in qs:
                    nc.tensor.matmul(
                        accq_t[q].rearrange("p a b -> p (a b)"),
                        ident[:], q2(main_h, q),
                        start=True, stop=False,
                        skip_group_check=True,
                    )

            def emit_mm2(e_t, cs, mbs, stop):
                for mb in mbs:
                    nc.tensor.matmul(
                        accq_t[mb // QT][:, mb % QT],
                        e_t[:, mb * P:(mb + 1) * P], cs[:],
                        start=False, stop=stop,
                        skip_group_check=True,
                    )

            def emit_dummy(n, w=2 * P):
                # HAM warm-keeper: matmuls into acc tile 2's upper half
                # (f32 cols 256+), disjoint from its staging lower half;
                # emit_ident start=True-overwrites it before any real read.
                dst = accq_t[2].rearrange("p a b -> p (a b)")
                for _ in range(n):
                    nc.tensor.matmul(
                        dst[:, 2 * P:2 * P + w],
                        dummy[:, 0:P], dummy[:, 0:w],
                        start=True, stop=True,
                        skip_group_check=True,
                    )

            # warm burst while the loads are in flight (PE otherwise idle
            # 7.4-12us): sustained MM activity flips HAM to 2.4GHz by
            # ~11us, so the real transposes and mm1(0) run at full rate.
            emit_dummy(16)

            # fp16 loads land directly in ctx_h/main_h -- no cast stage.
            # PE transposes gate straight on the DMA arrivals; ACT
            # carries the q0/ctx01/q2 evacuations (idle until exp(0)h0),
            # DVE carries q1/q3/ctx23.  Short dummy bursts keep HAM's MID
            # window from re-throttling between real matmul batches.
            # Staging map avoids same-tile cross-engine reader coupling:
            # main qk -> tile k lower half, ctx01 -> tile 3 upper, ctx23
            # -> tile 1 upper, dummies -> tile 2 upper.
            # ctx tiles 0-1 land first on SWDGE; transpose them before
            # T-q0 so mm1(0)'s stationary is never the prologue gate.
            pe_transpose_batch(ctx_h, ctxT, slice(0, 2), 3, QW, "scalar")
            pe_transpose_batch(main_h, mainT, slice(0, QT), 0, 0, "scalar")
            emit_dummy(3, P)
            pe_transpose_batch(main_h, mainT, slice(QT, 2 * QT), 1, 0)
            emit_dummy(3, P)
            pe_transpose_batch(main_h, mainT, slice(2 * QT, 3 * QT), 2, 0,
                               "scalar")
            emit_dummy(3, P)
            # first mm1 h0 (slot parity: h0 -> slot0, h1 -> slot1)
            emit_mm1(0, 0)
            pe_transpose_batch(main_h, mainT, slice(3 * QT, 4 * QT), 3, 0)
            emit_mm1(0, 1)
            pe_transpose_batch(ctx_h, ctxT, slice(2, 4), 1, QW)

            # ---- main loop (one-tile software pipeline) ----
            prev = None
            for ct in range(T):
                if ct < NQ - 1:
                    # ctxT quarters 1..3 ride the DMA xbar on the sync ring,
                    # which is idle during the loop; needed at tiles 4/8/12.
                    q = ct + 1
                    ts = slice(q * QT, (q + 1) * QT)
                    nc.sync.dma_start_transpose(ctxT[:, ts], q2(ctx_h, q))
                if ct == T - 2:
                    # warm the two store rings ~3us before the real stores
                    nc.sync.dma_start(out_lin[:, 0:16], dwarm[:])
                    nc.gpsimd.dma_start(out_lin[:, QW:QW + 16], dwarm[:])

                e_t = etile_pool.tile([P, S], BF16, tag="e")
                s_part = small.tile([P, 2], F32, tag="spart")
                nc.scalar.activation(e_t[:, 0:HALF], w_tiles[(ct, 0)][:], EXP)
                nc.vector.tensor_reduce(
                    s_part[:, 0:1], e_t[:, 0:HALF], axis=AX, op=ADD
                )
                # mm1(ct+1)h0 BEFORE mm2(ct-1) in the PE FIFO: mm2's gate
                # (the stats chain of ct-1) lands ~1.2us into the period,
                # a hair after mm1's gate (exp(ct)h0's slot release), so
                # the old order let mm2 head-of-line-block mm1 and push
                # exp(ct+1)h0 late by ~380ns on alternate periods.
                if ct + 1 < T:
                    emit_mm1(ct + 1, 0)
                if prev is not None:
                    emit_mm2(prev[0], prev[1], range(0, T // 2), stop=False)
                if prev is None and ct == 0:
                    # acc-init AFTER mm1(1)h0 in the PE queue: the cold
                    # N=512 identity matmuls otherwise sit in front of
                    # exp(1)'s inputs (measured ~1.4us of exp(1) stall);
                    # mm2(0) only needs them one full period later.
                    emit_ident([0, 1])
                nc.scalar.activation(
                    e_t[:, HALF:S], w_tiles[(ct, 1)][:], EXP,
                    accum_out=s_part[:, 1:2],
                )
                if ct + 1 < T:
                    emit_mm1(ct + 1, 1)
                if prev is not None:
                    emit_mm2(prev[0], prev[1], range(T // 2, T), stop=False)
                if prev is None and ct == 0:
                    emit_ident([2, 3])
                # stats: ctx_s = -context/S so acc accumulates main - weighted
                s_neg = small.tile([P, 1], F32, tag="ssum")
                nc.vector.tensor_scalar(
                    s_neg[:], s_part[:, 0:1], s_part[:, 1:2], -1.0, ADD, MULT
                )
                sinv = small.tile([P, 1], F32, tag="sinv")
                nc.vector.reciprocal(sinv[:], s_neg[:])
                ctx_s = small.tile([P, P], BF16, tag="ctxs")
                nc.vector.tensor_scalar_mul(ctx_s[:], ctx_h[:, ct], sinv[:])
                prev = (e_t, ctx_s)

            # ---- tail: ALL final mm2 blocks first (a PSUM read of acc
            # serializes later PE writes to it, so no mm2/copy interleave),
            # then bf16 evacs in ACT||DVE pairs -- the per-quarter acc
            # tiles make the cross-engine reads genuinely parallel.
            # Stores ride the sync + gpsimd queues so neither evac engine
            # queues a DMA issue behind its own copies.
            e_l, cs_l = prev
            emit_mm2(e_l, cs_l, range(T), stop=True)
            outs = []
            for q in range(NQ):
                out_sb = tailp.tile([P, QT, P], BF16, tag="outsb")
                outs.append(out_sb.rearrange("p a b -> p (a b)"))

            def accq(q):
                return accq_t[q].rearrange("p a b -> p (a b)")

            nc.scalar.copy(outs[0], accq(0))
            nc.vector.tensor_copy(outs[1], accq(1))
            nc.sync.dma_start(out_lin[:, 0 * QW:1 * QW], outs[0])
            nc.gpsimd.dma_start(out_lin[:, 1 * QW:2 * QW], outs[1])
            nc.scalar.copy(outs[2], accq(2))
            nc.vector.tensor_copy(outs[3], accq(3))
            nc.sync.dma_start(out_lin[:, 2 * QW:3 * QW], outs[2])
            nc.gpsimd.dma_start(out_lin[:, 3 * QW:4 * QW], outs[3])

    nc.compile()
    return nc


_RUNNER_CACHE = {}


def _get_runner(S, D):
    """Compile once and return a reusable jitted SPMD runner.

    run_bass_kernel_spmd re-jits (and re-runs the NEFF compiler) on every
    call, so repeated kernel() invocations would each pay minutes of
    compile; this builds the bass_exec + shard_map executable one time.
    """
    key = (S, D)
    if key in _RUNNER_CACHE:
        return _RUNNER_CACHE[key]

    import jax
    import concourse.mybir as _mybir
    from concourse.bass2jax import (
        _bass_exec_p,
        install_neuronx_cc_hook,
        partition_id_tensor,
    )
    from jax.sharding import Mesh, PartitionSpec
    from jax.experimental.shard_map import shard_map

    install_neuronx_cc_hook()
    nc = build_nc(S, D)

    part_name = nc.partition_id_tensor.name if nc.partition_id_tensor else None
    in_names, out_names, out_avals, zero_outs = [], [], [], []
    for alloc in nc.m.functions[0].allocations:
        if not isinstance(alloc, _mybir.MemoryLocationSet):
            continue
        name = alloc.memorylocations[0].name
        if alloc.kind == "ExternalInput":
            if name == part_name:
                continue
            in_names.append(name)
        elif alloc.kind == "ExternalOutput":
            out_names.append(name)
            shape = tuple(alloc.tensor_shape)
            dtype = _mybir.dt.np(alloc.dtype)
            out_avals.append(jax.core.ShapedArray(shape, dtype))
            zero_outs.append(np.zeros(shape, dtype))

    all_in = in_names + out_names + ([part_name] if part_name else [])

    def _body(*args):
        operands = list(args)
        if part_name is not None:
            operands.append(partition_id_tensor())
        outs = _bass_exec_p.bind(
            *operands,
            out_avals=tuple(out_avals),
            in_names=tuple(all_in),
            out_names=tuple(out_names),
            lowering_input_output_aliases=(),
            sim_require_finite=True,
            sim_require_nnan=True,
            nc=nc,
        )
        return tuple(outs)

    devices = jax.devices()[:N_CORES]
    mesh = Mesh(np.asarray(devices), ("core",))
    nin = len(in_names) + len(out_names)
    sharded = jax.jit(
        shard_map(
            _body,
            mesh=mesh,
            in_specs=(PartitionSpec("core"),) * nin,
            out_specs=(PartitionSpec("core"),) * len(out_names),
            check_rep=False,
        ),
        keep_unused=True,
    )
    zeros_cat = [np.concatenate([z] * N_CORES, axis=0) for z in zero_outs]
    _RUNNER_CACHE[key] = (sharded, in_names, out_names, zeros_cat)
    return _RUNNER_CACHE[key]


def kernel(context: np.ndarray, main: np.ndarray) -> np.ndarray:
    B, S, D = context.shape
    assert main.shape == (B, S, D) and B == N_CORES
    sharded, in_names, out_names, zeros_cat = _get_runner(S, D)
    feed = {
        "context": np.ascontiguousarray(context, dtype=np.float16).reshape(B * S, D),
        "main": np.ascontiguousarray(main, dtype=np.float16).reshape(B * S, D),
    }
    args = [feed[n] for n in in_names] + zeros_cat
    outs = sharded(*args)
    out = np.asarray(outs[out_names.index("out")])
    return out.reshape(B, S, D).astype(np.float32)

